# revision 10
# baseline (speedup 1.0000x reference)
"""Multi-head attention (B=4, S=2048, D=1024, H=16) on 8 trn2 cores.

Sharding: core c -> batch b = c//2, head-half = c%2 (8 heads = 512 dims).
Each core computes attention for its (batch, 8 heads) and a partial output
projection over its 512 d-features; the host sums the two partials per batch
and adds the (bo + bv @ Wo.T) constant row vector.

Device dataflow (per core, all shapes hardcoded):
  Phase A: QK^T projections into transposed layout Q^T/K^T [128d(2 heads), S]
           (bf16), V in [s, d] layout (f32r) with key-padding mask folded in
           and a mask column appended per head ([V'|m], 65 cols/head).
  Phase B: per head: S^T[k,q] = K^T.T @ Q^T tiles -> ACT exp(x/8) -> P^T
           (f32r); [num^T; denom] = [V'|m].T @ P^T accumulated over k-tiles;
           reciprocal of denom row, PE ones-broadcast, DVE multiply ->
           valsT [d, s] (f32r).
  Phase C: out[s,:] += valsT.T @ WoT accumulated over the 4 head-pair blocks.
"""

import numpy as np
from contextlib import ExitStack

import concourse.bacc as bacc
import concourse.tile as tile
import concourse.mybir as mybir
from concourse.bass_utils import run_bass_kernel_spmd

F32 = mybir.dt.float32
F32R = mybir.dt.float32r
BF16 = mybir.dt.bfloat16
EXP = mybir.ActivationFunctionType.Exp

S = 2048          # sequence length
D = 1024          # model dim
HD = 64           # head dim
NHL = 8           # heads per core
HP = 4            # head pairs per core (128 dims each)
DLOC = 512        # d-features per core
ET = D // 128     # 8 contraction tiles over D
ST = S // 128     # 16 s-tiles
QC = S // 512     # 4 query chunks of 512
KK = S // 128     # 16 key tiles of 128
VW = HD + 1       # V block width per head incl. mask column


def build_nc():
    nc = bacc.Bacc(None)
    xT = nc.dram_tensor("xT", [D, S], F32, kind="ExternalInput")
    wqT = nc.dram_tensor("wqT", [D, DLOC], F32, kind="ExternalInput")
    wkT = nc.dram_tensor("wkT", [D, DLOC], F32, kind="ExternalInput")
    wvT = nc.dram_tensor("wvT", [D, DLOC], F32, kind="ExternalInput")
    woT = nc.dram_tensor("woT", [DLOC, D], F32, kind="ExternalInput")
    bq = nc.dram_tensor("bq", [DLOC, 1], F32, kind="ExternalInput")
    bk = nc.dram_tensor("bk", [DLOC, 1], F32, kind="ExternalInput")
    msk = nc.dram_tensor("msk", [S, 1], F32, kind="ExternalInput")
    out = nc.dram_tensor("out", [S, D], F32, kind="ExternalOutput")

    with tile.TileContext(nc) as tc, ExitStack() as ctx:
        res = ctx.enter_context(tc.tile_pool(name="res", bufs=1))

        kt = [res.tile([128, S], F32R, tag=f"kt{i}", name=f"kt{i}") for i in range(HP)]
        vm = [res.tile([128, NHL * VW], F32R, tag=f"vm{i}", name=f"vm{i}") for i in range(KK)]
        valsT = [res.tile([128, S], F32R, tag=f"valsT{i}", name=f"valsT{i}") for i in range(HP)]
        # current-chunk Q^T tiles, rewritten every q-chunk (double-buffered)
        qtp = ctx.enter_context(tc.tile_pool(name="qtp", bufs=2))

        m_sb = res.tile([128, ST], F32, tag="m_sb")
        nc.sync.dma_start(out=m_sb, in_=msk.rearrange("(a p) o -> p (a o)", p=128))
        bq_sb = res.tile([128, HP], F32, tag="bq_sb")
        nc.sync.dma_start(out=bq_sb, in_=bq.rearrange("(a p) o -> p (a o)", p=128))
        bk_sb = res.tile([128, HP], F32, tag="bk_sb")
        nc.sync.dma_start(out=bk_sb, in_=bk.rearrange("(a p) o -> p (a o)", p=128))

        # ---------- Pass 1: K and V projections (stream xT once) ----------
        with tc.tile_pool(name="pa", bufs=1) as pa, \
             tc.tile_pool(name="xtp", bufs=10) as xtp, \
             tc.tile_pool(name="psA", bufs=4, space="PSUM") as psA:
            wk_sb = [pa.tile([128, DLOC], F32R, tag=f"wk{e}", name=f"wk{e}") for e in range(ET)]
            wv_sb = [pa.tile([128, DLOC], F32R, tag=f"wv{e}", name=f"wv{e}") for e in range(ET)]
            for e in range(ET):
                er = slice(e * 128, (e + 1) * 128)
                nc.sync.dma_start(out=wk_sb[e], in_=wkT[er, :].bitcast(F32R))
                nc.sync.dma_start(out=wv_sb[e], in_=wvT[er, :].bitcast(F32R))
            for qc in range(QC):
                cs = slice(qc * 512, (qc + 1) * 512)
                xts = [xtp.tile([128, 512], F32R, tag="xt", name=f"xt{qc}_{e}") for e in range(ET)]
                for e in range(ET):
                    nc.sync.dma_start(out=xts[e],
                                      in_=xT[e * 128:(e + 1) * 128, cs].bitcast(F32R))
                for hp in range(HP):
                    hcols = slice(hp * 128, (hp + 1) * 128)
                    psK = psA.tile([128, 512], F32, tag="psA")
                    for e in range(ET):
                        nc.tensor.matmul(psK, wk_sb[e][:, hcols], xts[e],
                                         start=(e == 0), stop=(e == ET - 1))
                    nc.vector.tensor_scalar_add(kt[hp][:, cs], psK, bk_sb[:, hp:hp + 1])
                for j in range(4):
                    st = qc * 4 + j
                    js = slice(j * 128, (j + 1) * 128)
                    psV = psA.tile([128, 512], F32, tag="psA")
                    for e in range(ET):
                        nc.tensor.matmul(psV, xts[e][:, js], wv_sb[e],
                                         start=(e == 0), stop=(e == ET - 1))
                    mc = m_sb[:, st:st + 1]
                    for h in range(NHL):
                        nc.vector.tensor_scalar_mul(
                            vm[st][:, h * VW:h * VW + HD],
                            psV[:, h * HD:(h + 1) * HD], mc)
                        nc.vector.tensor_copy(vm[st][:, h * VW + HD:h * VW + VW], mc)

        # ---- Pass 2: per q-chunk: Q projection + attention + out proj ----
        with tc.tile_pool(name="pb", bufs=1) as pb, \
             tc.tile_pool(name="xtq", bufs=8) as xtq, \
             tc.tile_pool(name="ptp", bufs=7) as ptp, \
             tc.tile_pool(name="sm", bufs=2) as sm, \
             tc.tile_pool(name="ob", bufs=2) as ob, \
             tc.tile_pool(name="psS", bufs=2, space="PSUM") as psSp, \
             tc.tile_pool(name="psO", bufs=2, space="PSUM") as psOp, \
             tc.tile_pool(name="psC", bufs=2, space="PSUM") as psCp:
            wq_sb = [pb.tile([128, DLOC], F32R, tag=f"wq{e}", name=f"wq{e}") for e in range(ET)]
            for e in range(ET):
                nc.sync.dma_start(out=wq_sb[e],
                                  in_=wqT[e * 128:(e + 1) * 128, :].bitcast(F32R))
            wo_sb = [pb.tile([128, D], F32R, tag=f"wo{i}", name=f"wo{i}") for i in range(HP)]
            for i in range(HP):
                nc.sync.dma_start(out=wo_sb[i],
                                  in_=woT[i * 128:(i + 1) * 128, :].bitcast(F32R))

            for qc in range(QC):
                cs = slice(qc * 512, (qc + 1) * 512)
                xts = [xtq.tile([128, 512], F32R, tag="xt2", name=f"x2{qc}_{e}") for e in range(ET)]
                for e in range(ET):
                    nc.sync.dma_start(out=xts[e],
                                      in_=xT[e * 128:(e + 1) * 128, cs].bitcast(F32R))
                qtc = [qtp.tile([128, 512], F32R, tag=f"qt{hp}", name=f"qt{qc}_{hp}")
                       for hp in range(HP)]
                for hp in range(HP):
                    hcols = slice(hp * 128, (hp + 1) * 128)
                    psQ = psSp.tile([128, 512], F32, tag="psS")
                    for e in range(ET):
                        nc.tensor.matmul(psQ, wq_sb[e][:, hcols], xts[e],
                                         start=(e == 0), stop=(e == ET - 1))
                    nc.vector.tensor_scalar_add(qtc[hp], psQ, bq_sb[:, hp:hp + 1])

                for hp in range(HP):
                    pts = [[], []]
                    for kp in range(KK // 2):
                        psS = [None, None]
                        for h2 in range(2):
                            psS[h2] = psSp.tile([128, 1024], F32, tag="psS",
                                                name=f"psS{qc}_{hp}_{kp}_{h2}")
                        for u in range(2):
                            kk = kp * 2 + u
                            ks = slice(kk * 128, (kk + 1) * 128)
                            for h2 in range(2):
                                hr = slice(h2 * 64, (h2 + 1) * 64)
                                nc.tensor.matmul(
                                    psS[h2][:, u * 512:(u + 1) * 512],
                                    kt[hp][hr, ks], qtc[hp][hr, :],
                                    start=True, stop=True)
                        for h2 in range(2):
                            pt = ptp.tile([128, 1024], F32R, tag="pt",
                                          name=f"pt{qc}_{hp}_{kp}_{h2}")
                            nc.scalar.activation(pt, psS[h2], EXP, scale=0.125)
                            pts[h2].append(pt)
                    for h2 in range(2):
                        h = hp * 2 + h2
                        hr = slice(h2 * 64, (h2 + 1) * 64)
                        psO = psOp.tile([128, 512], F32, tag="psO",
                                        name=f"psO{qc}_{hp}_{h2}")
                        for kp in range(KK // 2):
                            for u in range(2):
                                kk = kp * 2 + u
                                nc.tensor.matmul(
                                    psO[0:VW, :],
                                    vm[kk][:, h * VW:(h + 1) * VW],
                                    pts[h2][kp][:, u * 512:(u + 1) * 512],
                                    start=(kk == 0), stop=(kk == KK - 1))
                        dn = sm.tile([1, 512], F32, tag="dn")
                        nc.vector.tensor_copy(dn, psO[HD:VW, :])
                        dnb = sm.tile([64, 512], F32, tag="dnb")
                        nc.gpsimd.partition_broadcast(dnb, dn)
                        nc.vector.reciprocal(dnb, dnb)
                        nc.vector.tensor_mul(valsT[hp][hr, cs], psO[0:HD, :], dnb)

                for j in range(4):
                    st = qc * 4 + j
                    ss = slice(st * 128, (st + 1) * 128)
                    ot = ob.tile([128, D], F32, tag="ot", name=f"ot{st}")
                    for ec in range(2):
                        es = slice(ec * 512, (ec + 1) * 512)
                        psC = psCp.tile([128, 512], F32, tag="psC",
                                        name=f"psC{st}_{ec}")
                        for hp in range(HP):
                            nc.tensor.matmul(psC, valsT[hp][:, ss],
                                             wo_sb[hp][:, es],
                                             start=(hp == 0), stop=(hp == HP - 1))
                        nc.vector.tensor_copy(ot[:, es], psC)
                    nc.sync.dma_start(out=out[ss, :], in_=ot)

    nc.finalize()
    return nc


_NC_CACHE = None


def _get_nc():
    global _NC_CACHE
    if _NC_CACHE is None:
        _NC_CACHE = build_nc()
    return _NC_CACHE


def make_in_maps(x, mask, Wq, bq, Wk, bk, Wv, Wo):
    in_maps = []
    for c in range(8):
        b = c // 2
        dsl = slice((c % 2) * DLOC, (c % 2) * DLOC + DLOC)
        in_maps.append({
            "xT": np.ascontiguousarray(x[b].T, dtype=np.float32),
            "wqT": np.ascontiguousarray(Wq[dsl, :].T, dtype=np.float32),
            "wkT": np.ascontiguousarray(Wk[dsl, :].T, dtype=np.float32),
            "wvT": np.ascontiguousarray(Wv[dsl, :].T, dtype=np.float32),
            "woT": np.ascontiguousarray(Wo[:, dsl].T, dtype=np.float32),
            "bq": np.ascontiguousarray(bq[dsl], dtype=np.float32)[:, None],
            "bk": np.ascontiguousarray(bk[dsl], dtype=np.float32)[:, None],
            "msk": mask[b].astype(np.float32)[:, None],
        })
    return in_maps


def assemble(results, Wo, bo, bv):
    out = np.empty((4, S, D), dtype=np.float32)
    for b in range(4):
        out[b] = results[2 * b]["out"] + results[2 * b + 1]["out"]
    out += (bo + bv @ Wo.T).astype(np.float32)
    return out


def run(x, mask, Wq, bq, Wk, bk, Wv, bv, Wo, bo, trace=False):
    nc = _get_nc()
    in_maps = make_in_maps(x, mask, Wq, bq, Wk, bk, Wv, Wo)
    res = run_bass_kernel_spmd(nc, in_maps, list(range(8)), trace=trace)
    return assemble(res.results, Wo, bo, bv), res


def kernel(x, mask, Wq, bq, Wk, bk, Wv, bv, Wo, bo):
    out, _ = run(x, mask, Wq, bq, Wk, bk, Wv, bv, Wo, bo)
    return out


# revision 11
# speedup vs baseline: 1.0831x; 1.0831x over previous
"""Multi-head attention (B=4, S=2048, D=1024, H=16) on 8 trn2 cores.

Sharding: core c -> batch b = c//2, head-half = c%2 (8 heads = 512 dims).
Each core computes attention for its (batch, 8 heads) and a partial output
projection over its 512 d-features; the host sums the two partials per batch
and adds the (bo + bv @ Wo.T) constant row vector.

Device dataflow (per core, all shapes hardcoded):
  Phase A: QK^T projections into transposed layout Q^T/K^T [128d(2 heads), S]
           (bf16), V in [s, d] layout (f32r) with key-padding mask folded in
           and a mask column appended per head ([V'|m], 65 cols/head).
  Phase B: per head: S^T[k,q] = K^T.T @ Q^T tiles -> ACT exp(x/8) -> P^T
           (f32r); [num^T; denom] = [V'|m].T @ P^T accumulated over k-tiles;
           reciprocal of denom row, PE ones-broadcast, DVE multiply ->
           valsT [d, s] (f32r).
  Phase C: out[s,:] += valsT.T @ WoT accumulated over the 4 head-pair blocks.
"""

import numpy as np
from contextlib import ExitStack

import concourse.bacc as bacc
import concourse.tile as tile
import concourse.mybir as mybir
from concourse.bass_utils import run_bass_kernel_spmd

F32 = mybir.dt.float32
F32R = mybir.dt.float32r
BF16 = mybir.dt.bfloat16
EXP = mybir.ActivationFunctionType.Exp

S = 2048          # sequence length
D = 1024          # model dim
HD = 64           # head dim
NHL = 8           # heads per core
HP = 4            # head pairs per core (128 dims each)
DLOC = 512        # d-features per core
ET = D // 128     # 8 contraction tiles over D
ST = S // 128     # 16 s-tiles
QC = S // 512     # 4 query chunks of 512
KK = S // 128     # 16 key tiles of 128
VW = HD + 1       # V block width per head incl. mask column


def build_nc():
    nc = bacc.Bacc(None)
    xT = nc.dram_tensor("xT", [D, S], F32, kind="ExternalInput")
    wqT = nc.dram_tensor("wqT", [D, DLOC], F32, kind="ExternalInput")
    wkT = nc.dram_tensor("wkT", [D, DLOC], F32, kind="ExternalInput")
    wvT = nc.dram_tensor("wvT", [D, DLOC], F32, kind="ExternalInput")
    woT = nc.dram_tensor("woT", [DLOC, D], F32, kind="ExternalInput")
    bq = nc.dram_tensor("bq", [DLOC, 1], F32, kind="ExternalInput")
    bk = nc.dram_tensor("bk", [DLOC, 1], F32, kind="ExternalInput")
    msk = nc.dram_tensor("msk", [S, 1], F32, kind="ExternalInput")
    out = nc.dram_tensor("out", [S, D], F32, kind="ExternalOutput")

    with tile.TileContext(nc) as tc, ExitStack() as ctx:
        res = ctx.enter_context(tc.tile_pool(name="res", bufs=1))

        kt = [res.tile([128, S], F32R, tag=f"kt{i}", name=f"kt{i}") for i in range(HP)]
        vm = [res.tile([128, NHL * VW], F32R, tag=f"vm{i}", name=f"vm{i}") for i in range(KK)]
        valsT = [res.tile([128, S], F32R, tag=f"valsT{i}", name=f"valsT{i}") for i in range(HP)]
        # current-chunk Q^T tiles, rewritten every q-chunk (double-buffered)
        qtp = ctx.enter_context(tc.tile_pool(name="qtp", bufs=2))

        m_sb = res.tile([128, ST], F32, tag="m_sb")
        nc.sync.dma_start(out=m_sb, in_=msk.rearrange("(a p) o -> p (a o)", p=128))
        bq_sb = res.tile([128, HP], F32, tag="bq_sb")
        nc.sync.dma_start(out=bq_sb, in_=bq.rearrange("(a p) o -> p (a o)", p=128))
        bk_sb = res.tile([128, HP], F32, tag="bk_sb")
        nc.sync.dma_start(out=bk_sb, in_=bk.rearrange("(a p) o -> p (a o)", p=128))

        # ---------- Pass 1: K and V projections (stream xT once) ----------
        with tc.tile_pool(name="pa", bufs=1) as pa, \
             tc.tile_pool(name="xtp", bufs=10) as xtp, \
             tc.tile_pool(name="psA", bufs=4, space="PSUM") as psA:
            wk_sb = [pa.tile([128, DLOC], F32R, tag=f"wk{e}", name=f"wk{e}") for e in range(ET)]
            wv_sb = [pa.tile([128, DLOC], F32R, tag=f"wv{e}", name=f"wv{e}") for e in range(ET)]
            for e in range(ET):
                er = slice(e * 128, (e + 1) * 128)
                nc.sync.dma_start(out=wk_sb[e], in_=wkT[er, :].bitcast(F32R))
                nc.sync.dma_start(out=wv_sb[e], in_=wvT[er, :].bitcast(F32R))
            for qc in range(QC):
                cs = slice(qc * 512, (qc + 1) * 512)
                xts = [xtp.tile([128, 512], F32R, tag="xt", name=f"xt{qc}_{e}") for e in range(ET)]
                for e in range(ET):
                    nc.sync.dma_start(out=xts[e],
                                      in_=xT[e * 128:(e + 1) * 128, cs].bitcast(F32R))
                for hp in range(HP):
                    hcols = slice(hp * 128, (hp + 1) * 128)
                    psK = psA.tile([128, 512], F32, tag="psA")
                    for e in range(ET):
                        nc.tensor.matmul(psK, wk_sb[e][:, hcols], xts[e],
                                         start=(e == 0), stop=(e == ET - 1))
                    nc.vector.tensor_scalar_add(kt[hp][:, cs], psK, bk_sb[:, hp:hp + 1])
                for j in range(4):
                    st = qc * 4 + j
                    js = slice(j * 128, (j + 1) * 128)
                    psV = psA.tile([128, 512], F32, tag="psA")
                    for e in range(ET):
                        nc.tensor.matmul(psV, xts[e][:, js], wv_sb[e],
                                         start=(e == 0), stop=(e == ET - 1))
                    mc = m_sb[:, st:st + 1]
                    for h in range(NHL):
                        nc.vector.tensor_scalar_mul(
                            vm[st][:, h * VW:h * VW + HD],
                            psV[:, h * HD:(h + 1) * HD], mc)
                        nc.vector.tensor_copy(vm[st][:, h * VW + HD:h * VW + VW], mc)

        # ---- Pass 2: per q-chunk: Q projection + attention + out proj ----
        with tc.tile_pool(name="pb", bufs=1) as pb, \
             tc.tile_pool(name="xtq", bufs=8) as xtq, \
             tc.tile_pool(name="ptp", bufs=7) as ptp, \
             tc.tile_pool(name="sm", bufs=2) as sm, \
             tc.tile_pool(name="ob", bufs=2) as ob, \
             tc.tile_pool(name="psS", bufs=2, space="PSUM") as psSp, \
             tc.tile_pool(name="psO", bufs=2, space="PSUM") as psOp, \
             tc.tile_pool(name="psC", bufs=2, space="PSUM") as psCp:
            wq_sb = [pb.tile([128, DLOC], F32R, tag=f"wq{e}", name=f"wq{e}") for e in range(ET)]
            for e in range(ET):
                nc.sync.dma_start(out=wq_sb[e],
                                  in_=wqT[e * 128:(e + 1) * 128, :].bitcast(F32R))
            wo_sb = [pb.tile([128, D], F32R, tag=f"wo{i}", name=f"wo{i}") for i in range(HP)]
            for i in range(HP):
                nc.sync.dma_start(out=wo_sb[i],
                                  in_=woT[i * 128:(i + 1) * 128, :].bitcast(F32R))

            for qc in range(QC):
                cs = slice(qc * 512, (qc + 1) * 512)
                xts = [xtq.tile([128, 512], F32R, tag="xt2", name=f"x2{qc}_{e}") for e in range(ET)]
                for e in range(ET):
                    nc.sync.dma_start(out=xts[e],
                                      in_=xT[e * 128:(e + 1) * 128, cs].bitcast(F32R))
                qtc = [qtp.tile([128, 512], F32R, tag=f"qt{hp}", name=f"qt{qc}_{hp}")
                       for hp in range(HP)]
                for hp in range(HP):
                    hcols = slice(hp * 128, (hp + 1) * 128)
                    psQ = psSp.tile([128, 512], F32, tag="psS")
                    for e in range(ET):
                        nc.tensor.matmul(psQ, wq_sb[e][:, hcols], xts[e],
                                         start=(e == 0), stop=(e == ET - 1))
                    nc.vector.tensor_scalar_add(qtc[hp], psQ, bq_sb[:, hp:hp + 1])

                for hp in range(HP):
                    pts = [[], []]
                    for kp in range(KK // 2):
                        psS = [None, None]
                        for h2 in range(2):
                            psS[h2] = psSp.tile([128, 1024], F32, tag="psS",
                                                name=f"psS{qc}_{hp}_{kp}_{h2}")
                        for u in range(2):
                            kk = kp * 2 + u
                            ks = slice(kk * 128, (kk + 1) * 128)
                            for h2 in range(2):
                                hr = slice(h2 * 64, (h2 + 1) * 64)
                                nc.tensor.matmul(
                                    psS[h2][:, u * 512:(u + 1) * 512],
                                    kt[hp][hr, ks], qtc[hp][hr, :],
                                    start=True, stop=True)
                        for h2 in range(2):
                            pt = ptp.tile([128, 1024], F32R, tag="pt",
                                          name=f"pt{qc}_{hp}_{kp}_{h2}")
                            nc.scalar.activation(pt, psS[h2], EXP, scale=0.125)
                            pts[h2].append(pt)
                    for h2 in range(2):
                        h = hp * 2 + h2
                        hr = slice(h2 * 64, (h2 + 1) * 64)
                        psO = psOp.tile([128, 512], F32, tag="psO",
                                        name=f"psO{qc}_{hp}_{h2}")
                        for kp in range(KK // 2):
                            for u in range(2):
                                kk = kp * 2 + u
                                nc.tensor.matmul(
                                    psO[0:VW, :],
                                    vm[kk][:, h * VW:(h + 1) * VW],
                                    pts[h2][kp][:, u * 512:(u + 1) * 512],
                                    start=(kk == 0), stop=(kk == KK - 1))
                        dn = sm.tile([1, 512], F32, tag="dn")
                        nc.vector.tensor_copy(dn, psO[HD:VW, :])
                        dnb = sm.tile([64, 512], F32, tag="dnb")
                        nc.gpsimd.partition_broadcast(dnb, dn)
                        nc.vector.reciprocal_approx_fast(out=dnb, in_=dnb)
                        nc.vector.tensor_mul(valsT[hp][hr, cs], psO[0:HD, :], dnb)

                for j in range(4):
                    st = qc * 4 + j
                    ss = slice(st * 128, (st + 1) * 128)
                    ot = ob.tile([128, D], F32, tag="ot", name=f"ot{st}")
                    for ec in range(2):
                        es = slice(ec * 512, (ec + 1) * 512)
                        psC = psCp.tile([128, 512], F32, tag="psC",
                                        name=f"psC{st}_{ec}")
                        for hp in range(HP):
                            nc.tensor.matmul(psC, valsT[hp][:, ss],
                                             wo_sb[hp][:, es],
                                             start=(hp == 0), stop=(hp == HP - 1))
                        nc.vector.tensor_copy(ot[:, es], psC)
                    nc.sync.dma_start(out=out[ss, :], in_=ot)

    nc.finalize()
    return nc


_NC_CACHE = None


def _get_nc():
    global _NC_CACHE
    if _NC_CACHE is None:
        _NC_CACHE = build_nc()
    return _NC_CACHE


def make_in_maps(x, mask, Wq, bq, Wk, bk, Wv, Wo):
    in_maps = []
    for c in range(8):
        b = c // 2
        dsl = slice((c % 2) * DLOC, (c % 2) * DLOC + DLOC)
        in_maps.append({
            "xT": np.ascontiguousarray(x[b].T, dtype=np.float32),
            "wqT": np.ascontiguousarray(Wq[dsl, :].T, dtype=np.float32),
            "wkT": np.ascontiguousarray(Wk[dsl, :].T, dtype=np.float32),
            "wvT": np.ascontiguousarray(Wv[dsl, :].T, dtype=np.float32),
            "woT": np.ascontiguousarray(Wo[:, dsl].T, dtype=np.float32),
            "bq": np.ascontiguousarray(bq[dsl], dtype=np.float32)[:, None],
            "bk": np.ascontiguousarray(bk[dsl], dtype=np.float32)[:, None],
            "msk": mask[b].astype(np.float32)[:, None],
        })
    return in_maps


def assemble(results, Wo, bo, bv):
    out = np.empty((4, S, D), dtype=np.float32)
    for b in range(4):
        out[b] = results[2 * b]["out"] + results[2 * b + 1]["out"]
    out += (bo + bv @ Wo.T).astype(np.float32)
    return out


def run(x, mask, Wq, bq, Wk, bk, Wv, bv, Wo, bo, trace=False):
    nc = _get_nc()
    in_maps = make_in_maps(x, mask, Wq, bq, Wk, bk, Wv, Wo)
    res = run_bass_kernel_spmd(nc, in_maps, list(range(8)), trace=trace)
    return assemble(res.results, Wo, bo, bv), res


def kernel(x, mask, Wq, bq, Wk, bk, Wv, bv, Wo, bo):
    out, _ = run(x, mask, Wq, bq, Wk, bk, Wv, bv, Wo, bo)
    return out


# revision 12
# speedup vs baseline: 1.0908x; 1.0071x over previous
"""Multi-head attention (B=4, S=2048, D=1024, H=16) on 8 trn2 cores.

Sharding: core c -> batch b = c//2, head-half = c%2 (8 heads = 512 dims).
Each core computes attention for its (batch, 8 heads) and a partial output
projection over its 512 d-features; the host sums the two partials per batch
and adds the (bo + bv @ Wo.T) constant row vector.

Device dataflow (per core, all shapes hardcoded):
  Phase A: QK^T projections into transposed layout Q^T/K^T [128d(2 heads), S]
           (bf16), V in [s, d] layout (f32r) with key-padding mask folded in
           and a mask column appended per head ([V'|m], 65 cols/head).
  Phase B: per head: S^T[k,q] = K^T.T @ Q^T tiles -> ACT exp(x/8) -> P^T
           (f32r); [num^T; denom] = [V'|m].T @ P^T accumulated over k-tiles;
           reciprocal of denom row, PE ones-broadcast, DVE multiply ->
           valsT [d, s] (f32r).
  Phase C: out[s,:] += valsT.T @ WoT accumulated over the 4 head-pair blocks.
"""

import numpy as np
from contextlib import ExitStack

import concourse.bacc as bacc
import concourse.tile as tile
import concourse.mybir as mybir
from concourse.bass_utils import run_bass_kernel_spmd

F32 = mybir.dt.float32
F32R = mybir.dt.float32r
BF16 = mybir.dt.bfloat16
EXP = mybir.ActivationFunctionType.Exp

S = 2048          # sequence length
D = 1024          # model dim
HD = 64           # head dim
NHL = 8           # heads per core
HP = 4            # head pairs per core (128 dims each)
DLOC = 512        # d-features per core
ET = D // 128     # 8 contraction tiles over D
ST = S // 128     # 16 s-tiles
QC = S // 512     # 4 query chunks of 512
KK = S // 128     # 16 key tiles of 128
VW = HD + 1       # V block width per head incl. mask column


def build_nc():
    nc = bacc.Bacc(None)
    xT = nc.dram_tensor("xT", [D, S], F32, kind="ExternalInput")
    wqT = nc.dram_tensor("wqT", [D, DLOC], F32, kind="ExternalInput")
    wkT = nc.dram_tensor("wkT", [D, DLOC], F32, kind="ExternalInput")
    wvT = nc.dram_tensor("wvT", [D, DLOC], F32, kind="ExternalInput")
    woT = nc.dram_tensor("woT", [DLOC, D], F32, kind="ExternalInput")
    bq = nc.dram_tensor("bq", [DLOC, 1], F32, kind="ExternalInput")
    bk = nc.dram_tensor("bk", [DLOC, 1], F32, kind="ExternalInput")
    msk = nc.dram_tensor("msk", [S, 1], F32, kind="ExternalInput")
    out = nc.dram_tensor("out", [S, D], F32, kind="ExternalOutput")

    with tile.TileContext(nc) as tc, ExitStack() as ctx:
        res = ctx.enter_context(tc.tile_pool(name="res", bufs=1))

        kt = [res.tile([128, S], F32R, tag=f"kt{i}", name=f"kt{i}") for i in range(HP)]
        vm = [res.tile([128, NHL * VW], F32R, tag=f"vm{i}", name=f"vm{i}") for i in range(KK)]
        valsT = [res.tile([128, S], F32R, tag=f"valsT{i}", name=f"valsT{i}") for i in range(HP)]
        # current-chunk Q^T tiles, rewritten every q-chunk (double-buffered)
        qtp = ctx.enter_context(tc.tile_pool(name="qtp", bufs=2))

        m_sb = res.tile([128, ST], F32, tag="m_sb")
        nc.sync.dma_start(out=m_sb, in_=msk.rearrange("(a p) o -> p (a o)", p=128))
        bq_sb = res.tile([128, HP], F32, tag="bq_sb")
        nc.sync.dma_start(out=bq_sb, in_=bq.rearrange("(a p) o -> p (a o)", p=128))
        bk_sb = res.tile([128, HP], F32, tag="bk_sb")
        nc.sync.dma_start(out=bk_sb, in_=bk.rearrange("(a p) o -> p (a o)", p=128))

        # Q / O weights prefetched during pass 1
        pb = ctx.enter_context(tc.tile_pool(name="pb", bufs=1))
        wq_sb = [pb.tile([128, DLOC], F32R, tag=f"wq{e}", name=f"wq{e}") for e in range(ET)]
        wo_sb = [pb.tile([128, D], F32R, tag=f"wo{i}", name=f"wo{i}") for i in range(HP)]

        # ---------- Pass 1: K and V projections (stream xT once) ----------
        with tc.tile_pool(name="pa", bufs=1) as pa, \
             tc.tile_pool(name="xtp", bufs=10) as xtp, \
             tc.tile_pool(name="psA", bufs=4, space="PSUM") as psA:
            wk_sb = [pa.tile([128, DLOC], F32R, tag=f"wk{e}", name=f"wk{e}") for e in range(ET)]
            wv_sb = [pa.tile([128, DLOC], F32R, tag=f"wv{e}", name=f"wv{e}") for e in range(ET)]
            for e in range(ET):
                er = slice(e * 128, (e + 1) * 128)
                nc.sync.dma_start(out=wk_sb[e], in_=wkT[er, :].bitcast(F32R))
                nc.sync.dma_start(out=wv_sb[e], in_=wvT[er, :].bitcast(F32R))
            for e in range(ET):
                nc.sync.dma_start(out=wq_sb[e],
                                  in_=wqT[e * 128:(e + 1) * 128, :].bitcast(F32R))
            for i in range(HP):
                nc.sync.dma_start(out=wo_sb[i],
                                  in_=woT[i * 128:(i + 1) * 128, :].bitcast(F32R))
            for qc in range(QC):
                cs = slice(qc * 512, (qc + 1) * 512)
                xts = [xtp.tile([128, 512], F32R, tag="xt", name=f"xt{qc}_{e}") for e in range(ET)]
                for e in range(ET):
                    nc.sync.dma_start(out=xts[e],
                                      in_=xT[e * 128:(e + 1) * 128, cs].bitcast(F32R))
                for hp in range(HP):
                    hcols = slice(hp * 128, (hp + 1) * 128)
                    psK = psA.tile([128, 512], F32, tag="psA")
                    for e in range(ET):
                        nc.tensor.matmul(psK, wk_sb[e][:, hcols], xts[e],
                                         start=(e == 0), stop=(e == ET - 1))
                    nc.vector.tensor_scalar_add(kt[hp][:, cs], psK, bk_sb[:, hp:hp + 1])
                for j in range(4):
                    st = qc * 4 + j
                    js = slice(j * 128, (j + 1) * 128)
                    psV = psA.tile([128, 512], F32, tag="psA")
                    for e in range(ET):
                        nc.tensor.matmul(psV, xts[e][:, js], wv_sb[e],
                                         start=(e == 0), stop=(e == ET - 1))
                    mc = m_sb[:, st:st + 1]
                    for h in range(NHL):
                        nc.vector.tensor_scalar_mul(
                            vm[st][:, h * VW:h * VW + HD],
                            psV[:, h * HD:(h + 1) * HD], mc)
                        nc.vector.tensor_copy(vm[st][:, h * VW + HD:h * VW + VW], mc)

        # ---- Pass 2: per q-chunk: Q projection + attention + out proj ----
        with tc.tile_pool(name="xtq", bufs=8) as xtq, \
             tc.tile_pool(name="ptp", bufs=7) as ptp, \
             tc.tile_pool(name="sm", bufs=2) as sm, \
             tc.tile_pool(name="ob", bufs=2) as ob, \
             tc.tile_pool(name="psS", bufs=2, space="PSUM") as psSp, \
             tc.tile_pool(name="psO", bufs=2, space="PSUM") as psOp, \
             tc.tile_pool(name="psC", bufs=2, space="PSUM") as psCp:
            for qc in range(QC):
                cs = slice(qc * 512, (qc + 1) * 512)
                xts = [xtq.tile([128, 512], F32R, tag="xt2", name=f"x2{qc}_{e}") for e in range(ET)]
                for e in range(ET):
                    nc.sync.dma_start(out=xts[e],
                                      in_=xT[e * 128:(e + 1) * 128, cs].bitcast(F32R))
                qtc = [qtp.tile([128, 512], F32R, tag=f"qt{hp}", name=f"qt{qc}_{hp}")
                       for hp in range(HP)]
                for hp in range(HP):
                    hcols = slice(hp * 128, (hp + 1) * 128)
                    psQ = psSp.tile([128, 512], F32, tag="psS")
                    for e in range(ET):
                        nc.tensor.matmul(psQ, wq_sb[e][:, hcols], xts[e],
                                         start=(e == 0), stop=(e == ET - 1))
                    nc.vector.tensor_scalar_add(qtc[hp], psQ, bq_sb[:, hp:hp + 1])

                for hp in range(HP):
                    pts = [[], []]
                    for kp in range(KK // 2):
                        psS = [None, None]
                        for h2 in range(2):
                            psS[h2] = psSp.tile([128, 1024], F32, tag="psS",
                                                name=f"psS{qc}_{hp}_{kp}_{h2}")
                        for u in range(2):
                            kk = kp * 2 + u
                            ks = slice(kk * 128, (kk + 1) * 128)
                            for h2 in range(2):
                                hr = slice(h2 * 64, (h2 + 1) * 64)
                                nc.tensor.matmul(
                                    psS[h2][:, u * 512:(u + 1) * 512],
                                    kt[hp][hr, ks], qtc[hp][hr, :],
                                    start=True, stop=True)
                        for h2 in range(2):
                            pt = ptp.tile([128, 1024], F32R, tag="pt",
                                          name=f"pt{qc}_{hp}_{kp}_{h2}")
                            nc.scalar.activation(pt, psS[h2], EXP, scale=0.125)
                            pts[h2].append(pt)
                    for h2 in range(2):
                        h = hp * 2 + h2
                        hr = slice(h2 * 64, (h2 + 1) * 64)
                        psO = psOp.tile([128, 512], F32, tag="psO",
                                        name=f"psO{qc}_{hp}_{h2}")
                        for kp in range(KK // 2):
                            for u in range(2):
                                kk = kp * 2 + u
                                nc.tensor.matmul(
                                    psO[0:VW, :],
                                    vm[kk][:, h * VW:(h + 1) * VW],
                                    pts[h2][kp][:, u * 512:(u + 1) * 512],
                                    start=(kk == 0), stop=(kk == KK - 1))
                        dn = sm.tile([1, 512], F32, tag="dn")
                        nc.vector.tensor_copy(dn, psO[HD:VW, :])
                        dnb = sm.tile([64, 512], F32, tag="dnb")
                        nc.gpsimd.partition_broadcast(dnb, dn)
                        nc.vector.reciprocal_approx_fast(out=dnb, in_=dnb)
                        nc.vector.tensor_mul(valsT[hp][hr, cs], psO[0:HD, :], dnb)

                for j in range(4):
                    st = qc * 4 + j
                    ss = slice(st * 128, (st + 1) * 128)
                    ot = ob.tile([128, D], F32, tag="ot", name=f"ot{st}")
                    for ec in range(2):
                        es = slice(ec * 512, (ec + 1) * 512)
                        psC = psCp.tile([128, 512], F32, tag="psC",
                                        name=f"psC{st}_{ec}")
                        for hp in range(HP):
                            nc.tensor.matmul(psC, valsT[hp][:, ss],
                                             wo_sb[hp][:, es],
                                             start=(hp == 0), stop=(hp == HP - 1))
                        nc.vector.tensor_copy(ot[:, es], psC)
                    nc.sync.dma_start(out=out[ss, :], in_=ot)

    nc.finalize()
    return nc


_NC_CACHE = None


def _get_nc():
    global _NC_CACHE
    if _NC_CACHE is None:
        _NC_CACHE = build_nc()
    return _NC_CACHE


def make_in_maps(x, mask, Wq, bq, Wk, bk, Wv, Wo):
    in_maps = []
    for c in range(8):
        b = c // 2
        dsl = slice((c % 2) * DLOC, (c % 2) * DLOC + DLOC)
        in_maps.append({
            "xT": np.ascontiguousarray(x[b].T, dtype=np.float32),
            "wqT": np.ascontiguousarray(Wq[dsl, :].T, dtype=np.float32),
            "wkT": np.ascontiguousarray(Wk[dsl, :].T, dtype=np.float32),
            "wvT": np.ascontiguousarray(Wv[dsl, :].T, dtype=np.float32),
            "woT": np.ascontiguousarray(Wo[:, dsl].T, dtype=np.float32),
            "bq": np.ascontiguousarray(bq[dsl], dtype=np.float32)[:, None],
            "bk": np.ascontiguousarray(bk[dsl], dtype=np.float32)[:, None],
            "msk": mask[b].astype(np.float32)[:, None],
        })
    return in_maps


def assemble(results, Wo, bo, bv):
    out = np.empty((4, S, D), dtype=np.float32)
    for b in range(4):
        out[b] = results[2 * b]["out"] + results[2 * b + 1]["out"]
    out += (bo + bv @ Wo.T).astype(np.float32)
    return out


def run(x, mask, Wq, bq, Wk, bk, Wv, bv, Wo, bo, trace=False):
    nc = _get_nc()
    in_maps = make_in_maps(x, mask, Wq, bq, Wk, bk, Wv, Wo)
    res = run_bass_kernel_spmd(nc, in_maps, list(range(8)), trace=trace)
    return assemble(res.results, Wo, bo, bv), res


def kernel(x, mask, Wq, bq, Wk, bk, Wv, bv, Wo, bo):
    out, _ = run(x, mask, Wq, bq, Wk, bk, Wv, bv, Wo, bo)
    return out


# revision 13
# speedup vs baseline: 1.1374x; 1.0427x over previous
"""Multi-head attention (B=4, S=2048, D=1024, H=16) on 8 trn2 cores.

Sharding: core c -> batch b = c//2, head-half = c%2 (8 heads = 512 dims).
Each core computes attention for its (batch, 8 heads) and a partial output
projection over its 512 d-features; the host sums the two partials per batch
and adds the (bo + bv @ Wo.T) constant row vector.

Device dataflow (per core, all shapes hardcoded):
  Phase A: QK^T projections into transposed layout Q^T/K^T [128d(2 heads), S]
           (bf16), V in [s, d] layout (f32r) with key-padding mask folded in
           and a mask column appended per head ([V'|m], 65 cols/head).
  Phase B: per head: S^T[k,q] = K^T.T @ Q^T tiles -> ACT exp(x/8) -> P^T
           (f32r); [num^T; denom] = [V'|m].T @ P^T accumulated over k-tiles;
           reciprocal of denom row, PE ones-broadcast, DVE multiply ->
           valsT [d, s] (f32r).
  Phase C: out[s,:] += valsT.T @ WoT accumulated over the 4 head-pair blocks.
"""

import numpy as np
from contextlib import ExitStack

import concourse.bacc as bacc
import concourse.tile as tile
import concourse.mybir as mybir
from concourse.bass_utils import run_bass_kernel_spmd

F32 = mybir.dt.float32
F32R = mybir.dt.float32r
BF16 = mybir.dt.bfloat16
EXP = mybir.ActivationFunctionType.Exp

S = 2048          # sequence length
D = 1024          # model dim
HD = 64           # head dim
NHL = 8           # heads per core
HP = 4            # head pairs per core (128 dims each)
DLOC = 512        # d-features per core
ET = D // 128     # 8 contraction tiles over D
ST = S // 128     # 16 s-tiles
QC = S // 512     # 4 query chunks of 512
KK = S // 128     # 16 key tiles of 128
VW = HD + 1       # V block width per head incl. mask column


def build_nc():
    nc = bacc.Bacc(None)
    xT = nc.dram_tensor("xT", [D, S], F32, kind="ExternalInput")
    wqT = nc.dram_tensor("wqT", [D, DLOC], F32, kind="ExternalInput")
    wkT = nc.dram_tensor("wkT", [D, DLOC], F32, kind="ExternalInput")
    wvT = nc.dram_tensor("wvT", [D, DLOC], F32, kind="ExternalInput")
    woT = nc.dram_tensor("woT", [DLOC, D], F32, kind="ExternalInput")
    bq = nc.dram_tensor("bq", [DLOC, 1], F32, kind="ExternalInput")
    bk = nc.dram_tensor("bk", [DLOC, 1], F32, kind="ExternalInput")
    msk = nc.dram_tensor("msk", [S, 1], F32, kind="ExternalInput")
    out = nc.dram_tensor("out", [S, D], F32, kind="ExternalOutput")

    with tile.TileContext(nc) as tc, ExitStack() as ctx:
        res = ctx.enter_context(tc.tile_pool(name="res", bufs=1))

        kt = [res.tile([128, S], F32R, tag=f"kt{i}", name=f"kt{i}") for i in range(HP)]
        vm = [res.tile([128, NHL * VW], F32R, tag=f"vm{i}", name=f"vm{i}") for i in range(KK)]
        valsT = [res.tile([128, S], F32R, tag=f"valsT{i}", name=f"valsT{i}") for i in range(HP)]
        # current-chunk Q^T tiles, rewritten every q-chunk (double-buffered)
        qtp = ctx.enter_context(tc.tile_pool(name="qtp", bufs=1))

        m_sb = res.tile([128, ST], F32, tag="m_sb")
        nc.sync.dma_start(out=m_sb, in_=msk.rearrange("(a p) o -> p (a o)", p=128))
        bq_sb = res.tile([128, HP], F32, tag="bq_sb")
        nc.sync.dma_start(out=bq_sb, in_=bq.rearrange("(a p) o -> p (a o)", p=128))
        bk_sb = res.tile([128, HP], F32, tag="bk_sb")
        nc.sync.dma_start(out=bk_sb, in_=bk.rearrange("(a p) o -> p (a o)", p=128))

        # Q / O weights prefetched during pass 1
        pb = ctx.enter_context(tc.tile_pool(name="pb", bufs=1))
        wq_sb = [pb.tile([128, DLOC], F32R, tag=f"wq{e}", name=f"wq{e}") for e in range(ET)]
        wo_sb = [pb.tile([128, D], F32R, tag=f"wo{i}", name=f"wo{i}") for i in range(HP)]

        # ---------- Pass 1: K and V projections (stream xT once) ----------
        with tc.tile_pool(name="pa", bufs=1) as pa, \
             tc.tile_pool(name="xtp", bufs=10) as xtp, \
             tc.tile_pool(name="psA", bufs=4, space="PSUM") as psA:
            wk_sb = [pa.tile([128, DLOC], F32R, tag=f"wk{e}", name=f"wk{e}") for e in range(ET)]
            wv_sb = [pa.tile([128, DLOC], F32R, tag=f"wv{e}", name=f"wv{e}") for e in range(ET)]
            for e in range(ET):
                nc.sync.dma_start(out=wk_sb[e],
                                  in_=wkT[e * 128:(e + 1) * 128, :].bitcast(F32R))
            for qc in range(QC):
                cs = slice(qc * 512, (qc + 1) * 512)
                xts = [xtp.tile([128, 512], F32R, tag="xt", name=f"xt{qc}_{e}") for e in range(ET)]
                for e in range(ET):
                    nc.sync.dma_start(out=xts[e],
                                      in_=xT[e * 128:(e + 1) * 128, cs].bitcast(F32R))
                if qc == 0:
                    for e in range(ET):
                        nc.sync.dma_start(out=wv_sb[e],
                                          in_=wvT[e * 128:(e + 1) * 128, :].bitcast(F32R))
                elif qc == 1:
                    for e in range(ET):
                        nc.sync.dma_start(out=wq_sb[e],
                                          in_=wqT[e * 128:(e + 1) * 128, :].bitcast(F32R))
                    for i in range(HP):
                        nc.sync.dma_start(out=wo_sb[i],
                                          in_=woT[i * 128:(i + 1) * 128, :].bitcast(F32R))
                for hp in range(HP):
                    hcols = slice(hp * 128, (hp + 1) * 128)
                    psK = psA.tile([128, 512], F32, tag="psA")
                    for e in range(ET):
                        nc.tensor.matmul(psK, wk_sb[e][:, hcols], xts[e],
                                         start=(e == 0), stop=(e == ET - 1))
                    nc.vector.tensor_scalar_add(kt[hp][:, cs], psK, bk_sb[:, hp:hp + 1])
                for j in range(4):
                    st = qc * 4 + j
                    js = slice(j * 128, (j + 1) * 128)
                    psV = psA.tile([128, 512], F32, tag="psA")
                    for e in range(ET):
                        nc.tensor.matmul(psV, xts[e][:, js], wv_sb[e],
                                         start=(e == 0), stop=(e == ET - 1))
                    mc = m_sb[:, st:st + 1]
                    for h in range(NHL):
                        nc.vector.tensor_scalar_mul(
                            vm[st][:, h * VW:h * VW + HD],
                            psV[:, h * HD:(h + 1) * HD], mc)
                        nc.vector.tensor_copy(vm[st][:, h * VW + HD:h * VW + VW], mc)

        # ---- Pass 2: per q-chunk: Q projection + attention + out proj ----
        with tc.tile_pool(name="xtq", bufs=12) as xtq, \
             tc.tile_pool(name="ptp", bufs=7) as ptp, \
             tc.tile_pool(name="sm", bufs=2) as sm, \
             tc.tile_pool(name="ob", bufs=2) as ob, \
             tc.tile_pool(name="psS", bufs=2, space="PSUM") as psSp, \
             tc.tile_pool(name="psO", bufs=2, space="PSUM") as psOp, \
             tc.tile_pool(name="psC", bufs=2, space="PSUM") as psCp:
            for qc in range(QC):
                cs = slice(qc * 512, (qc + 1) * 512)
                xts = [xtq.tile([128, 512], F32R, tag="xt2", name=f"x2{qc}_{e}") for e in range(ET)]
                for e in range(ET):
                    nc.sync.dma_start(out=xts[e],
                                      in_=xT[e * 128:(e + 1) * 128, cs].bitcast(F32R))
                qtc = [qtp.tile([128, 512], F32R, tag=f"qt{hp}", name=f"qt{qc}_{hp}")
                       for hp in range(HP)]
                for hp in range(HP):
                    hcols = slice(hp * 128, (hp + 1) * 128)
                    psQ = psSp.tile([128, 512], F32, tag="psS")
                    for e in range(ET):
                        nc.tensor.matmul(psQ, wq_sb[e][:, hcols], xts[e],
                                         start=(e == 0), stop=(e == ET - 1))
                    nc.vector.tensor_scalar_add(qtc[hp], psQ, bq_sb[:, hp:hp + 1])

                for hp in range(HP):
                    pts = [[], []]
                    for kp in range(KK // 2):
                        psS = [None, None]
                        for h2 in range(2):
                            psS[h2] = psSp.tile([128, 1024], F32, tag="psS",
                                                name=f"psS{qc}_{hp}_{kp}_{h2}")
                        for u in range(2):
                            kk = kp * 2 + u
                            ks = slice(kk * 128, (kk + 1) * 128)
                            for h2 in range(2):
                                hr = slice(h2 * 64, (h2 + 1) * 64)
                                nc.tensor.matmul(
                                    psS[h2][:, u * 512:(u + 1) * 512],
                                    kt[hp][hr, ks], qtc[hp][hr, :],
                                    start=True, stop=True)
                        for h2 in range(2):
                            pt = ptp.tile([128, 1024], F32R, tag="pt",
                                          name=f"pt{qc}_{hp}_{kp}_{h2}")
                            nc.scalar.activation(pt, psS[h2], EXP, scale=0.125)
                            pts[h2].append(pt)
                    for h2 in range(2):
                        h = hp * 2 + h2
                        hr = slice(h2 * 64, (h2 + 1) * 64)
                        psO = psOp.tile([128, 512], F32, tag="psO",
                                        name=f"psO{qc}_{hp}_{h2}")
                        for kp in range(KK // 2):
                            for u in range(2):
                                kk = kp * 2 + u
                                nc.tensor.matmul(
                                    psO[0:VW, :],
                                    vm[kk][:, h * VW:(h + 1) * VW],
                                    pts[h2][kp][:, u * 512:(u + 1) * 512],
                                    start=(kk == 0), stop=(kk == KK - 1))
                        dn = sm.tile([1, 512], F32, tag="dn")
                        nc.vector.tensor_copy(dn, psO[HD:VW, :])
                        dnb = sm.tile([64, 512], F32, tag="dnb")
                        nc.gpsimd.partition_broadcast(dnb, dn)
                        nc.vector.reciprocal_approx_fast(out=dnb, in_=dnb)
                        nc.vector.tensor_mul(valsT[hp][hr, cs], psO[0:HD, :], dnb)

                for j in range(4):
                    st = qc * 4 + j
                    ss = slice(st * 128, (st + 1) * 128)
                    ot = ob.tile([128, D], F32, tag="ot", name=f"ot{st}")
                    for ec in range(2):
                        es = slice(ec * 512, (ec + 1) * 512)
                        psC = psCp.tile([128, 512], F32, tag="psC",
                                        name=f"psC{st}_{ec}")
                        for hp in range(HP):
                            nc.tensor.matmul(psC, valsT[hp][:, ss],
                                             wo_sb[hp][:, es],
                                             start=(hp == 0), stop=(hp == HP - 1))
                        nc.vector.tensor_copy(ot[:, es], psC)
                    nc.sync.dma_start(out=out[ss, :], in_=ot)

    nc.finalize()
    return nc


_NC_CACHE = None


def _get_nc():
    global _NC_CACHE
    if _NC_CACHE is None:
        _NC_CACHE = build_nc()
    return _NC_CACHE


def make_in_maps(x, mask, Wq, bq, Wk, bk, Wv, Wo):
    in_maps = []
    for c in range(8):
        b = c // 2
        dsl = slice((c % 2) * DLOC, (c % 2) * DLOC + DLOC)
        in_maps.append({
            "xT": np.ascontiguousarray(x[b].T, dtype=np.float32),
            "wqT": np.ascontiguousarray(Wq[dsl, :].T, dtype=np.float32),
            "wkT": np.ascontiguousarray(Wk[dsl, :].T, dtype=np.float32),
            "wvT": np.ascontiguousarray(Wv[dsl, :].T, dtype=np.float32),
            "woT": np.ascontiguousarray(Wo[:, dsl].T, dtype=np.float32),
            "bq": np.ascontiguousarray(bq[dsl], dtype=np.float32)[:, None],
            "bk": np.ascontiguousarray(bk[dsl], dtype=np.float32)[:, None],
            "msk": mask[b].astype(np.float32)[:, None],
        })
    return in_maps


def assemble(results, Wo, bo, bv):
    out = np.empty((4, S, D), dtype=np.float32)
    for b in range(4):
        out[b] = results[2 * b]["out"] + results[2 * b + 1]["out"]
    out += (bo + bv @ Wo.T).astype(np.float32)
    return out


def run(x, mask, Wq, bq, Wk, bk, Wv, bv, Wo, bo, trace=False):
    nc = _get_nc()
    in_maps = make_in_maps(x, mask, Wq, bq, Wk, bk, Wv, Wo)
    res = run_bass_kernel_spmd(nc, in_maps, list(range(8)), trace=trace)
    return assemble(res.results, Wo, bo, bv), res


def kernel(x, mask, Wq, bq, Wk, bk, Wv, bv, Wo, bo):
    out, _ = run(x, mask, Wq, bq, Wk, bk, Wv, bv, Wo, bo)
    return out


# revision 14
# speedup vs baseline: 1.1527x; 1.0135x over previous
"""Multi-head attention (B=4, S=2048, D=1024, H=16) on 8 trn2 cores.

Sharding: core c -> batch b = c//2, head-half = c%2 (8 heads = 512 dims).
Each core computes attention for its (batch, 8 heads) and a partial output
projection over its 512 d-features; the host sums the two partials per batch
and adds the (bo + bv @ Wo.T) constant row vector.

Device dataflow (per core, all shapes hardcoded):
  Phase A: QK^T projections into transposed layout Q^T/K^T [128d(2 heads), S]
           (bf16), V in [s, d] layout (f32r) with key-padding mask folded in
           and a mask column appended per head ([V'|m], 65 cols/head).
  Phase B: per head: S^T[k,q] = K^T.T @ Q^T tiles -> ACT exp(x/8) -> P^T
           (f32r); [num^T; denom] = [V'|m].T @ P^T accumulated over k-tiles;
           reciprocal of denom row, PE ones-broadcast, DVE multiply ->
           valsT [d, s] (f32r).
  Phase C: out[s,:] += valsT.T @ WoT accumulated over the 4 head-pair blocks.
"""

import numpy as np
from contextlib import ExitStack

import concourse.bacc as bacc
import concourse.tile as tile
import concourse.mybir as mybir
from concourse.bass_utils import run_bass_kernel_spmd

F32 = mybir.dt.float32
F32R = mybir.dt.float32r
BF16 = mybir.dt.bfloat16
EXP = mybir.ActivationFunctionType.Exp

S = 2048          # sequence length
D = 1024          # model dim
HD = 64           # head dim
NHL = 8           # heads per core
HP = 4            # head pairs per core (128 dims each)
DLOC = 512        # d-features per core
ET = D // 128     # 8 contraction tiles over D
ST = S // 128     # 16 s-tiles
QC = S // 512     # 4 query chunks of 512
KK = S // 128     # 16 key tiles of 128
VW = HD + 1       # V block width per head incl. mask column


def build_nc():
    nc = bacc.Bacc(None)
    xT = nc.dram_tensor("xT", [D, S], F32, kind="ExternalInput")
    wqT = nc.dram_tensor("wqT", [D, DLOC], F32, kind="ExternalInput")
    wkT = nc.dram_tensor("wkT", [D, DLOC], F32, kind="ExternalInput")
    wvT = nc.dram_tensor("wvT", [D, DLOC], F32, kind="ExternalInput")
    woT = nc.dram_tensor("woT", [DLOC, D], F32, kind="ExternalInput")
    bq = nc.dram_tensor("bq", [DLOC, 1], F32, kind="ExternalInput")
    bk = nc.dram_tensor("bk", [DLOC, 1], F32, kind="ExternalInput")
    msk = nc.dram_tensor("msk", [S, 1], F32, kind="ExternalInput")
    out = nc.dram_tensor("out", [S, D], F32, kind="ExternalOutput")

    with tile.TileContext(nc) as tc, ExitStack() as ctx:
        res = ctx.enter_context(tc.tile_pool(name="res", bufs=1))

        kt = [res.tile([128, S], F32R, tag=f"kt{i}", name=f"kt{i}") for i in range(HP)]
        vm = [res.tile([128, NHL * VW], F32R, tag=f"vm{i}", name=f"vm{i}") for i in range(KK)]
        valsT = [res.tile([128, S], F32R, tag=f"valsT{i}", name=f"valsT{i}") for i in range(HP)]
        # current-chunk Q^T tiles, rewritten every q-chunk (double-buffered)
        qtp = ctx.enter_context(tc.tile_pool(name="qtp", bufs=1))

        m_sb = res.tile([128, ST], F32, tag="m_sb")
        nc.sync.dma_start(out=m_sb, in_=msk.rearrange("(a p) o -> p (a o)", p=128))
        bq_sb = res.tile([128, HP], F32, tag="bq_sb")
        nc.sync.dma_start(out=bq_sb, in_=bq.rearrange("(a p) o -> p (a o)", p=128))
        bk_sb = res.tile([128, HP], F32, tag="bk_sb")
        nc.sync.dma_start(out=bk_sb, in_=bk.rearrange("(a p) o -> p (a o)", p=128))

        # Q / O weights prefetched during pass 1
        pb = ctx.enter_context(tc.tile_pool(name="pb", bufs=1))
        wq_sb = [pb.tile([128, DLOC], F32R, tag=f"wq{e}", name=f"wq{e}") for e in range(ET)]
        wo_sb = [pb.tile([128, D], F32R, tag=f"wo{i}", name=f"wo{i}") for i in range(HP)]

        # ---------- Pass 1: K and V projections (stream xT once) ----------
        with tc.tile_pool(name="pa", bufs=1) as pa, \
             tc.tile_pool(name="xtp", bufs=10) as xtp, \
             tc.tile_pool(name="psA", bufs=4, space="PSUM") as psA:
            wk_sb = [pa.tile([128, DLOC], F32R, tag=f"wk{e}", name=f"wk{e}") for e in range(ET)]
            wv_sb = [pa.tile([128, DLOC], F32R, tag=f"wv{e}", name=f"wv{e}") for e in range(ET)]
            for e in range(ET):
                nc.sync.dma_start(out=wk_sb[e],
                                  in_=wkT[e * 128:(e + 1) * 128, :].bitcast(F32R))
            for qc in range(QC):
                cs = slice(qc * 512, (qc + 1) * 512)
                xts = [xtp.tile([128, 512], F32R, tag="xt", name=f"xt{qc}_{e}") for e in range(ET)]
                for e in range(ET):
                    nc.sync.dma_start(out=xts[e],
                                      in_=xT[e * 128:(e + 1) * 128, cs].bitcast(F32R))
                if qc == 0:
                    for e in range(ET):
                        nc.sync.dma_start(out=wv_sb[e],
                                          in_=wvT[e * 128:(e + 1) * 128, :].bitcast(F32R))
                elif qc == 1:
                    for e in range(ET):
                        nc.sync.dma_start(out=wq_sb[e],
                                          in_=wqT[e * 128:(e + 1) * 128, :].bitcast(F32R))
                    for i in range(HP):
                        nc.sync.dma_start(out=wo_sb[i],
                                          in_=woT[i * 128:(i + 1) * 128, :].bitcast(F32R))
                for hp in range(HP):
                    hcols = slice(hp * 128, (hp + 1) * 128)
                    psK = psA.tile([128, 512], F32, tag="psA")
                    for e in range(ET):
                        nc.tensor.matmul(psK, wk_sb[e][:, hcols], xts[e],
                                         start=(e == 0), stop=(e == ET - 1))
                    nc.vector.tensor_scalar_add(kt[hp][:, cs], psK, bk_sb[:, hp:hp + 1])
                for j in range(4):
                    st = qc * 4 + j
                    js = slice(j * 128, (j + 1) * 128)
                    psV = psA.tile([128, 512], F32, tag="psA")
                    for e in range(ET):
                        nc.tensor.matmul(psV, xts[e][:, js], wv_sb[e],
                                         start=(e == 0), stop=(e == ET - 1))
                    mc = m_sb[:, st:st + 1]
                    for h in range(NHL):
                        nc.vector.tensor_scalar_mul(
                            vm[st][:, h * VW:h * VW + HD],
                            psV[:, h * HD:(h + 1) * HD], mc)
                        nc.gpsimd.tensor_copy(out=vm[st][:, h * VW + HD:h * VW + VW], in_=mc)

        # ---- Pass 2: per q-chunk: Q projection + attention + out proj ----
        with tc.tile_pool(name="xtq", bufs=12) as xtq, \
             tc.tile_pool(name="ptp", bufs=7) as ptp, \
             tc.tile_pool(name="sm", bufs=2) as sm, \
             tc.tile_pool(name="ob", bufs=2) as ob, \
             tc.tile_pool(name="psS", bufs=3, space="PSUM") as psSp, \
             tc.tile_pool(name="psO", bufs=2, space="PSUM") as psOp:
            for qc in range(QC):
                cs = slice(qc * 512, (qc + 1) * 512)
                xts = [xtq.tile([128, 512], F32R, tag="xt2", name=f"x2{qc}_{e}") for e in range(ET)]
                for e in range(ET):
                    nc.sync.dma_start(out=xts[e],
                                      in_=xT[e * 128:(e + 1) * 128, cs].bitcast(F32R))
                qtc = [qtp.tile([128, 512], F32R, tag=f"qt{hp}", name=f"qt{qc}_{hp}")
                       for hp in range(HP)]
                for hp in range(HP):
                    hcols = slice(hp * 128, (hp + 1) * 128)
                    psQ = psSp.tile([128, 512], F32, tag="psS")
                    for e in range(ET):
                        nc.tensor.matmul(psQ, wq_sb[e][:, hcols], xts[e],
                                         start=(e == 0), stop=(e == ET - 1))
                    nc.vector.tensor_scalar_add(qtc[hp], psQ, bq_sb[:, hp:hp + 1])

                for hp in range(HP):
                    pts = [[], []]
                    for kp in range(KK // 2):
                        psS = [None, None]
                        for h2 in range(2):
                            psS[h2] = psSp.tile([128, 1024], F32, tag="psS",
                                                name=f"psS{qc}_{hp}_{kp}_{h2}")
                        for u in range(2):
                            kk = kp * 2 + u
                            ks = slice(kk * 128, (kk + 1) * 128)
                            for h2 in range(2):
                                hr = slice(h2 * 64, (h2 + 1) * 64)
                                nc.tensor.matmul(
                                    psS[h2][:, u * 512:(u + 1) * 512],
                                    kt[hp][hr, ks], qtc[hp][hr, :],
                                    start=True, stop=True)
                        for h2 in range(2):
                            pt = ptp.tile([128, 1024], F32R, tag="pt",
                                          name=f"pt{qc}_{hp}_{kp}_{h2}")
                            nc.scalar.activation(pt, psS[h2], EXP, scale=0.125)
                            pts[h2].append(pt)
                    for h2 in range(2):
                        h = hp * 2 + h2
                        hr = slice(h2 * 64, (h2 + 1) * 64)
                        psO = psOp.tile([128, 512], F32, tag="psO",
                                        name=f"psO{qc}_{hp}_{h2}")
                        for kp in range(KK // 2):
                            for u in range(2):
                                kk = kp * 2 + u
                                nc.tensor.matmul(
                                    psO[0:VW, :],
                                    vm[kk][:, h * VW:(h + 1) * VW],
                                    pts[h2][kp][:, u * 512:(u + 1) * 512],
                                    start=(kk == 0), stop=(kk == KK - 1))
                        dn = sm.tile([1, 512], F32, tag="dn")
                        nc.vector.tensor_copy(dn, psO[HD:VW, :])
                        dnb = sm.tile([64, 512], F32, tag="dnb")
                        nc.gpsimd.partition_broadcast(dnb, dn)
                        nc.vector.reciprocal_approx_fast(out=dnb, in_=dnb)
                        nc.vector.tensor_mul(valsT[hp][hr, cs], psO[0:HD, :], dnb)

                for j in range(4):
                    st = qc * 4 + j
                    ss = slice(st * 128, (st + 1) * 128)
                    ot = ob.tile([128, D], F32, tag="ot", name=f"ot{st}")
                    for ec in range(2):
                        es = slice(ec * 512, (ec + 1) * 512)
                        psC = psOp.tile([128, 512], F32, tag="psO",
                                        name=f"psC{st}_{ec}")
                        for hp in range(HP):
                            nc.tensor.matmul(psC, valsT[hp][:, ss],
                                             wo_sb[hp][:, es],
                                             start=(hp == 0), stop=(hp == HP - 1))
                        nc.vector.tensor_copy(ot[:, es], psC)
                    nc.sync.dma_start(out=out[ss, :], in_=ot)

    nc.finalize()
    return nc


_NC_CACHE = None


def _get_nc():
    global _NC_CACHE
    if _NC_CACHE is None:
        _NC_CACHE = build_nc()
    return _NC_CACHE


def make_in_maps(x, mask, Wq, bq, Wk, bk, Wv, Wo):
    in_maps = []
    for c in range(8):
        b = c // 2
        dsl = slice((c % 2) * DLOC, (c % 2) * DLOC + DLOC)
        in_maps.append({
            "xT": np.ascontiguousarray(x[b].T, dtype=np.float32),
            "wqT": np.ascontiguousarray(Wq[dsl, :].T, dtype=np.float32),
            "wkT": np.ascontiguousarray(Wk[dsl, :].T, dtype=np.float32),
            "wvT": np.ascontiguousarray(Wv[dsl, :].T, dtype=np.float32),
            "woT": np.ascontiguousarray(Wo[:, dsl].T, dtype=np.float32),
            "bq": np.ascontiguousarray(bq[dsl], dtype=np.float32)[:, None],
            "bk": np.ascontiguousarray(bk[dsl], dtype=np.float32)[:, None],
            "msk": mask[b].astype(np.float32)[:, None],
        })
    return in_maps


def assemble(results, Wo, bo, bv):
    out = np.empty((4, S, D), dtype=np.float32)
    for b in range(4):
        out[b] = results[2 * b]["out"] + results[2 * b + 1]["out"]
    out += (bo + bv @ Wo.T).astype(np.float32)
    return out


def run(x, mask, Wq, bq, Wk, bk, Wv, bv, Wo, bo, trace=False):
    nc = _get_nc()
    in_maps = make_in_maps(x, mask, Wq, bq, Wk, bk, Wv, Wo)
    res = run_bass_kernel_spmd(nc, in_maps, list(range(8)), trace=trace)
    return assemble(res.results, Wo, bo, bv), res


def kernel(x, mask, Wq, bq, Wk, bk, Wv, bv, Wo, bo):
    out, _ = run(x, mask, Wq, bq, Wk, bk, Wv, bv, Wo, bo)
    return out
